# revision 1
# baseline (speedup 1.0000x reference)
"""Causal multi-head attention (B=4, S=2048, H=16, D=64, E=1024) on 8 TRN2 cores.

Sharding: data-parallel over batch (4) x tensor-parallel over heads (2 groups
of 8). Each core computes, for its (batch, head-group):
    q/k/v projections -> causal softmax attention -> output projection
and returns a partial [S, E] output (Wr row-split); the host adds the two
partials per batch.

All matmul operands are float32r (~TF32 precision at full PE rate).
Attention runs in the transposed layout (keys/head-dims on partitions) so no
on-chip transposes are needed; V carries an extra ones-column so the attn@V
matmul also emits the softmax denominators (output row 64).
"""

import numpy as np

import concourse.bacc as bacc
import concourse.bass as bass
import concourse.mybir as mybir
import concourse.tile as tile
from concourse.bass_utils import run_bass_kernel_spmd

HEADS = 16
HD = 64
EMB = 1024
B, S = 4, 2048
SCALE = 1.0 / 8.0
NCORES = 8
HPC = HEADS // 2          # heads per core (8)
GW = HPC * HD             # head-group width (512)

F32 = mybir.dt.float32
F32R = mybir.dt.float32r
EXP = mybir.ActivationFunctionType.Exp

NQC = 4                   # query chunks of 512
QW = 512                  # query chunk width
NKB = S // 128            # key blocks of 128 (16)
NEC = EMB // 128          # emb chunks (8)
NSB = S // 128            # seq blocks (16)


def build():
    nc = bacc.Bacc("TRN2", target_bir_lowering=False, debug=False)

    xt_d = nc.dram_tensor("xt", [EMB, S], F32R, kind="ExternalInput")
    # wq/wk pre-swizzled on host to [hp, p, e, n] so per-(hp) DMA is contiguous
    wq_d = nc.dram_tensor("wq", [4, 128, NEC, 128], F32R, kind="ExternalInput")
    wk_d = nc.dram_tensor("wk", [4, 128, NEC, 128], F32R, kind="ExternalInput")
    wv_d = nc.dram_tensor("wv", [EMB, GW], F32R, kind="ExternalInput")
    wr_d = nc.dram_tensor("wr", [GW, EMB], F32R, kind="ExternalInput")
    # consts: [:,0:128] causal tri mask, [:,128:256] ones, [:,256:640] zeros
    cn_d = nc.dram_tensor("consts", [128, 640], F32R, kind="ExternalInput")
    y_d = nc.dram_tensor("y", [S, EMB], F32, kind="ExternalOutput")

    with tile.TileContext(nc) as tc, nc.allow_low_precision(reason="f32r attn"):
        with (
            tc.tile_pool(name="persist", bufs=1) as pp,
            tc.tile_pool(name="qtp", bufs=1) as pq,
            tc.tile_pool(name="outp", bufs=1) as po,
            tc.tile_pool(name="attn", bufs=2) as pa,
            tc.tile_pool(name="recp", bufs=2) as prc,
            tc.tile_pool(name="bcp", bufs=2) as pbc,
            tc.tile_pool(name="ysb", bufs=1) as pyb,
            tc.tile_pool(name="ps_mm", bufs=1, space="PSUM") as ps_mm,
            tc.tile_pool(name="ps_y", bufs=1, space="PSUM") as ps_y,
            tc.tile_pool(name="ps_att", bufs=2, space="PSUM") as ps_att,
        ):
            kt = pp.tile([128, NQC, S], F32R, tag="kt")
            v = pp.tile([128, NKB, HPC, HD + 1], F32R, tag="v")
            wr = pp.tile([128, 4, EMB], F32R, tag="wr")
            mo = pp.tile([128, 640], F32R, tag="consts")
            mask = mo[:, 0:128]
            ones = mo[0:1, 128:192]
            zeros = mo[:, 256:640]

            nc.sync.dma_start(mo[:], cn_d.ap())
            nc.sync.dma_start(wr[:], wr_d.ap().rearrange("(c p) n -> p c n", p=128))
            # ones column of v (softmax denominator trick)
            nc.sync.dma_start(v[:, :, :, HD], cn_d.ap()[:, 128:256])

            qtiles = {}

            with (
                tc.tile_pool(name="proj", bufs=1) as pj,
                tc.tile_pool(name="wql", bufs=1) as pwq,
                tc.tile_pool(name="wkl", bufs=1) as pwk,
            ):
                xt = pj.tile([128, NEC, S], F32R, tag="xt")
                wv = pj.tile([128, NEC, GW], F32R, tag="wv")

                for e in range(NEC):
                    nc.sync.dma_start(xt[:, e, :], xt_d.ap()[e * 128:(e + 1) * 128, :])
                nc.sync.dma_start(wv[:], wv_d.ap().rearrange("(c p) n -> p c n", p=128))

                # ---- P1: v = x @ Wv, natural layout [seq, head, 64] ----
                for sb in range(NSB):
                    ps = ps_mm.tile([128, GW], F32, tag="mm")
                    for e in range(NEC):
                        nc.tensor.matmul(
                            ps[:], xt[:, e, sb * 128:(sb + 1) * 128], wv[:, e, :],
                            start=(e == 0), stop=(e == NEC - 1),
                        )
                    nc.any.tensor_copy(
                        v[:, sb, :, 0:HD],
                        ps[:].rearrange("p (h d) -> p h d", d=HD),
                    )

                # ---- P2: qT (per query chunk) and kT head-pair tiles ----
                for c in range(NQC):
                    csl = slice(c * QW, (c + 1) * QW)
                    qtile = pq.tile([128, NQC, QW], F32R, tag="qt")
                    qtiles[c] = qtile
                    for dst, wsrc, pool in ((qtile, wq_d, pwq), (kt, wk_d, pwk)):
                        for hp in range(4):
                            wc = pool.tile([128, NEC, 128], F32R, tag="w")
                            nc.sync.dma_start(wc[:], wsrc.ap()[hp])
                            ps = ps_mm.tile([128, QW], F32, tag="mm")
                            for e in range(NEC):
                                nc.tensor.matmul(
                                    ps[:], wc[:, e, :], xt[:, e, csl],
                                    start=(e == 0), stop=(e == NEC - 1),
                                )
                            if dst is qtile:
                                nc.any.tensor_copy(qtile[:, hp, :], ps[:])
                            else:
                                nc.any.tensor_copy(kt[:, hp, csl], ps[:])

            # ---- P3/P4: attention + output projection per query chunk ----
            for qc in range(NQC):
                kbmax = 4 * (qc + 1)
                qtile = qtiles[qc]
                outtc = po.tile([128, NQC, QW], F32R, tag="outt")
                for h in range(HPC):
                    hp, ho = h // 2, (h % 2) * HD
                    out_ps = ps_att.tile([HD + 1, QW], F32, tag="out")
                    for g in range(kbmax // 2):
                        sc = ps_att.tile([128, 2, QW], F32, tag="sc")
                        at = pa.tile([128, 2, QW], F32R, tag="at")
                        for s_ in range(2):
                            kb = 2 * g + s_
                            # scoresT block [keys, queries]
                            nc.tensor.matmul(
                                sc[:, s_, :],
                                kt[ho:ho + HD, hp, kb * 128:(kb + 1) * 128],
                                qtile[ho:ho + HD, hp, :],
                                start=True, stop=True,
                            )
                        nc.scalar.activation(at[:], sc[:], EXP)
                        for s_ in range(2):
                            kb = 2 * g + s_
                            j = kb - 4 * qc
                            if j >= 0:  # diagonal block: causal mask
                                if j > 0:
                                    nc.vector.tensor_copy(
                                        at[:, s_, 0:j * 128], zeros[:, 0:j * 128])
                                nc.vector.tensor_mul(
                                    at[:, s_, j * 128:(j + 1) * 128],
                                    at[:, s_, j * 128:(j + 1) * 128],
                                    mask,
                                )
                            nc.tensor.matmul(
                                out_ps[:],
                                v[:, kb, h, :],
                                at[:, s_, :],
                                start=(kb == 0), stop=(kb == kbmax - 1),
                            )
                    # rows 0..63 = (attn@v).T numerator, row 64 = denom
                    rec = prc.tile([1, QW], F32R, tag="rec")
                    nc.vector.reciprocal(rec[0:1, :], out_ps[HD:HD + 1, :])
                    bct = ps_att.tile([128, 2, QW], F32, tag="sc")
                    bc_ps = bct[0:HD, 0, :]
                    nc.tensor.matmul(bc_ps, ones, rec[0:1, :],
                                     start=True, stop=True)
                    bc = pbc.tile([HD, QW], F32R, tag="bcs")
                    nc.any.tensor_copy(bc[:], bc_ps)
                    nc.vector.tensor_mul(
                        outtc[ho:ho + HD, hp, :], out_ps[0:HD, :], bc[:],
                    )

                # ---- P4: y rows for this query chunk ----
                for sbl in range(4):
                    sb = qc * 4 + sbl
                    ysb = pyb.tile([128, EMB], F32, tag="ysb")
                    for ncol in range(2):
                        ps = ps_y.tile([128, QW], F32, tag="ymm")
                        for hp in range(4):
                            nc.tensor.matmul(
                                ps[:],
                                outtc[:, hp, sbl * 128:(sbl + 1) * 128],
                                wr[:, hp, ncol * QW:(ncol + 1) * QW],
                                start=(hp == 0), stop=(hp == 3),
                            )
                        nc.any.tensor_copy(ysb[:, ncol * QW:(ncol + 1) * QW], ps[:])
                    nc.sync.dma_start(y_d.ap()[sb * 128:(sb + 1) * 128, :], ysb[:])

    nc.compile()
    return nc


_NC_CACHE = None


def _get_nc():
    global _NC_CACHE
    if _NC_CACHE is None:
        _NC_CACHE = build()
    return _NC_CACHE


def make_in_maps(x, Wq, Wk, Wv, Wr):
    x = np.ascontiguousarray(x, dtype=np.float32)
    Wq = np.asarray(Wq, dtype=np.float32)
    Wk = np.asarray(Wk, dtype=np.float32)
    Wv = np.asarray(Wv, dtype=np.float32)
    Wr = np.asarray(Wr, dtype=np.float32)

    consts = np.zeros((128, 640), dtype=np.float32)
    consts[:, 0:128] = np.triu(np.ones((128, 128), dtype=np.float32))
    consts[:, 128:256] = 1.0

    def swz(w):  # [1024, 512] -> [hp, p, e, n]
        return np.ascontiguousarray(
            w.reshape(NEC, 128, 4, 128).transpose(2, 1, 0, 3))

    in_maps = []
    for core in range(NCORES):
        b, g = divmod(core, 2)
        hs = slice(g * GW, (g + 1) * GW)
        in_maps.append({
            "xt": np.ascontiguousarray(x[b].T),
            "wq": swz(Wq[:, hs] * SCALE),
            "wk": swz(Wk[:, hs]),
            "wv": np.ascontiguousarray(Wv[:, hs]),
            "wr": np.ascontiguousarray(Wr[hs, :]),
            "consts": consts,
        })
    return in_maps


def kernel(x, Wq, Wk, Wv, Wr):
    in_maps = make_in_maps(x, Wq, Wk, Wv, Wr)
    nc = _get_nc()
    res = run_bass_kernel_spmd(nc, in_maps, core_ids=list(range(NCORES)))

    y = np.empty((B, S, EMB), dtype=np.float32)
    for b in range(B):
        y[b] = res.results[2 * b]["y"] + res.results[2 * b + 1]["y"]
    return y



# revision 8
# speedup vs baseline: 1.8784x; 1.8784x over previous
"""Causal multi-head attention (B=4, S=2048, H=16, D=64, E=1024) on 8 TRN2 cores.

Sharding: data-parallel over batch (4) x tensor-parallel over heads (2 groups
of 8). Each core computes, for its (batch, head-group):
    q/k/v projections -> causal softmax attention -> output projection
and returns a partial [S, E] output (Wr row-split); the host adds the two
partials per batch.

Engine plan (per core):
  - Inputs (x, Wq, Wk, Wv, Wr) are host-cast to bf16: halves the DMA and
    runs the projection matmuls at 1 cycle/row.
  - Scores per head pair run as two row-tiled concurrent matmuls (head-even
    on PE rows 0-63, head-odd on rows 64-127, outputs in different PSUM
    banks), so the K=64 contraction doesn't waste half the array.
  - exp() on ScalarE over [128, 2, 512] (both heads of a pair per key
    block), writing bf16 `at`; causal diagonal masked on VectorE.
  - attn@V with a ones-column on V so the softmax denominators fall out of
    the same matmul (output row 64); denominators inverted with the fast
    custom-DVE reciprocal (~5x faster than the iterative one) and
    broadcast to 64 rows via a tiny PE matmul.
  - Projection / output-projection work is woven into the attention loop as
    "filler" so the PE never stalls on ScalarE exp and the HAM clock gate
    stays warm.
"""

from collections import deque

import numpy as np

import concourse.bacc as bacc
import concourse.bass as bass
import concourse.mybir as mybir
import concourse.tile as tile
from concourse.bass_utils import run_bass_kernel_spmd
from concourse.dve_ops import RECIP_APPROX_FAST_CONSTS, RECIPROCAL_APPROX_FAST

HEADS = 16
HD = 64
EMB = 1024
B, S = 4, 2048
SCALE = 1.0 / 8.0
NCORES = 8
HPC = HEADS // 2          # heads per core (8)
GW = HPC * HD             # head-group width (512)

F32 = mybir.dt.float32
F32R = mybir.dt.float32r
BF16 = mybir.dt.bfloat16
EXP = mybir.ActivationFunctionType.Exp

NQC = 4                   # query chunks of 512
QW = 512                  # query chunk width
NEC = EMB // 128          # emb chunks (8)
NSB = S // 128            # seq blocks (16)


def build():
    nc = bacc.Bacc("TRN2", target_bir_lowering=False, debug=False)

    xt_d = nc.dram_tensor("xt", [EMB, S], BF16, kind="ExternalInput")
    # weights pre-swizzled on host to partition-major so every DMA is one
    # contiguous transfer
    wq_d = nc.dram_tensor("wq", [128, 4, NEC, 128], BF16, kind="ExternalInput")
    wk_d = nc.dram_tensor("wk", [128, 4, NEC, 128], BF16, kind="ExternalInput")
    wv_d = nc.dram_tensor("wv", [128, NEC, GW], BF16, kind="ExternalInput")
    wr_d = nc.dram_tensor("wr", [128, 4, EMB], BF16, kind="ExternalInput")
    # bf16 consts: [:,0:128] causal tri mask, [:,128:256] ones, [256:640] zeros
    cb_d = nc.dram_tensor("cb", [128, 640], BF16, kind="ExternalInput")
    # f32r ones row (lhsT of the reciprocal-broadcast matmul)
    cr_d = nc.dram_tensor("cr", [1, 64], F32R, kind="ExternalInput")
    y_d = nc.dram_tensor("y", [S, EMB], F32, kind="ExternalOutput")

    rc = RECIP_APPROX_FAST_CONSTS

    with tile.TileContext(nc) as tc, nc.allow_low_precision(reason="bf16 attn"):
        with (
            tc.tile_pool(name="persist", bufs=1) as pp,
            tc.tile_pool(name="outp", bufs=2) as po,
            tc.tile_pool(name="attn", bufs=3) as pa,
            tc.tile_pool(name="recp", bufs=2) as prc,
            tc.tile_pool(name="bcp", bufs=2) as pbc,
            tc.tile_pool(name="ysb", bufs=2) as pyb,
            tc.tile_pool(name="ps_sc", bufs=2, space="PSUM") as ps_sc,
            tc.tile_pool(name="ps_out", bufs=2, space="PSUM") as ps_out,
            tc.tile_pool(name="ps_wv", bufs=2, space="PSUM") as ps_wv,
        ):
            xt = pp.tile([128, NEC, S], BF16, tag="xt")
            wq = pp.tile([128, 4, NEC, 128], BF16, tag="wq")
            wk = pp.tile([128, 4, NEC, 128], BF16, tag="wk")
            wv = pp.tile([128, NEC, GW], BF16, tag="wv")
            wr = pp.tile([128, 4, EMB], BF16, tag="wr")
            kt = pp.tile([128, NQC, S], F32R, tag="kt")
            qt = pp.tile([128, NQC, NQC, QW], F32R, tag="qt")  # [*, c, hp, q]
            v = pp.tile([128, NSB, HPC, HD + 1], BF16, tag="v")
            cb = pp.tile([128, 640], BF16, tag="cb")
            cr = pp.tile([1, 64], F32R, tag="cr")
            tri = cb[:, 0:128]
            zeros = cb[:, 256:640]

            nc.sync.dma_start(cb[:], cb_d.ap())
            nc.sync.dma_start(cr[:], cr_d.ap())
            nc.sync.dma_start(wv[:], wv_d.ap())
            nc.sync.dma_start(wq[:], wq_d.ap())
            nc.sync.dma_start(wk[:], wk_d.ap())
            nc.sync.dma_start(wr[:], wr_d.ap())
            for e in range(NEC):
                nc.sync.dma_start(xt[:, e, :], xt_d.ap()[e * 128:(e + 1) * 128, :])
            # ones column of v (softmax denominator trick)
            nc.sync.dma_start(v[:, :, :, HD], cb_d.ap()[:, 128:256])

            # ---------------- weave units ----------------
            def p1_unit(sb, cp):
                ps = ps_wv.tile([128, GW], F32, tag="wv", name=f"p1_{sb}")
                for e in range(NEC):
                    nc.tensor.matmul(
                        ps[:], xt[:, e, sb * 128:(sb + 1) * 128], wv[:, e, :],
                        start=(e == 0), stop=(e == NEC - 1),
                    )
                cp(
                    v[:, sb, :, 0:HD],
                    ps[:].rearrange("p (h d) -> p h d", d=HD),
                )

            def p2_unit(c, hp, is_q, cp):
                w = wq if is_q else wk
                ps = ps_wv.tile([128, QW], F32, tag="wv",
                                name=f"p2_{c}_{hp}_{int(is_q)}")
                for e in range(NEC):
                    nc.tensor.matmul(
                        ps[:], w[:, hp, e, :], xt[:, e, c * QW:(c + 1) * QW],
                        start=(e == 0), stop=(e == NEC - 1),
                    )
                if is_q:
                    cp(qt[:, c, hp, :], ps[:])
                else:
                    cp(kt[:, hp, c * QW:(c + 1) * QW], ps[:])

            def p4_unit(outtc, qc, sbl, cp):
                sb = qc * 4 + sbl
                ysb = pyb.tile([128, EMB], F32, tag="ysb", name=f"ysb_{sb}")
                for ncol in range(2):
                    ps = ps_wv.tile([128, QW], F32, tag="wv",
                                    name=f"p4_{sb}_{ncol}")
                    for hp in range(4):
                        nc.tensor.matmul(
                            ps[:],
                            outtc[:, hp, sbl * 128:(sbl + 1) * 128],
                            wr[:, hp, ncol * QW:(ncol + 1) * QW],
                            start=(hp == 0), stop=(hp == 3),
                        )
                    cp(ysb[:, ncol * QW:(ncol + 1) * QW], ps[:])
                nc.sync.dma_start(y_d.ap()[sb * 128:(sb + 1) * 128, :], ysb[:])

            # ---------------- prefix: v sb0-3, q/k chunk 0 ----------------
            # (ScalarE is idle here, so PSUM->SBUF copies go to it)
            for sb in range(4):
                p1_unit(sb, nc.scalar.copy)
            for hp in range(4):
                p2_unit(0, hp, True, nc.scalar.copy)
                p2_unit(0, hp, False, nc.scalar.copy)

            # Filler queues consumed inside the attention loop (VectorE
            # copies there -- ScalarE is saturated by exp). `bulk` entries
            # carry a deadline qc: they MUST be emitted before attention(qc)
            # reads their output (dep tracking is program-order).  P4 units
            # get priority so outtc/PSUM slots recycle promptly.
            bulk = deque()
            for c in range(1, NQC):
                for sb in range(4 * c, 4 * c + 4):
                    bulk.append(
                        (c, lambda sb=sb: p1_unit(sb, nc.vector.tensor_copy)))
                for hp in range(4):
                    bulk.append((c, lambda c=c, hp=hp: p2_unit(
                        c, hp, True, nc.vector.tensor_copy)))
                    bulk.append((c, lambda c=c, hp=hp: p2_unit(
                        c, hp, False, nc.vector.tensor_copy)))
            p4q = deque()

            def pop_filler():
                if p4q:
                    p4q.popleft()()
                elif bulk:
                    bulk.popleft()[1]()

            def drain_due(qc):
                while bulk and bulk[0][0] <= qc:
                    bulk.popleft()[1]()

            # ---------------- attention + output projection ----------------
            for qc in range(NQC):
                drain_due(qc)
                kbmax = 4 * (qc + 1)
                outtc = po.tile([128, NQC, QW], BF16, tag="outt",
                                name=f"outt_{qc}")
                for hp in range(4):
                    outps = [
                        ps_out.tile([HD + 1, QW], F32, tag="out",
                                    name=f"o_{qc}_{hp}_{s}")
                        for s in range(2)
                    ]
                    for kb in range(kbmax):
                        sc = ps_sc.tile([128, 2, QW], F32, tag="sc",
                                        name=f"sc_{qc}_{hp}_{kb}")
                        for s_ in range(2):
                            ho = s_ * HD
                            nc.tensor.matmul(
                                sc[:, s_, :],
                                kt[ho:ho + HD, hp, kb * 128:(kb + 1) * 128],
                                qt[ho:ho + HD, qc, hp, :],
                                start=True, stop=True,
                            )
                        at = pa.tile([128, 2, QW], BF16, tag="at",
                                     name=f"at_{qc}_{hp}_{kb}")
                        nc.scalar.activation(at[:], sc[:], EXP)
                        j = kb - 4 * qc
                        if j >= 0:  # diagonal block: causal mask
                            for s_ in range(2):
                                if j > 0:
                                    nc.vector.tensor_copy(
                                        at[:, s_, 0:j * 128],
                                        zeros[:, 0:j * 128])
                                nc.vector.tensor_mul(
                                    at[:, s_, j * 128:(j + 1) * 128],
                                    at[:, s_, j * 128:(j + 1) * 128],
                                    tri,
                                )
                        for s_ in range(2):
                            nc.tensor.matmul(
                                outps[s_][:],
                                v[:, kb, 2 * hp + s_, :],
                                at[:, s_, :],
                                start=(kb == 0), stop=(kb == kbmax - 1),
                            )
                        if kb % 2 == 1:
                            pop_filler()

                    # epilogue: rows 0..63 = (attn@v).T numerator, row 64 =
                    # denominator.  The custom-DVE reciprocal can't remap the
                    # partition base, so hop through a partition-0 SBUF tile.
                    for s_ in range(2):
                        ho = s_ * HD
                        den = prc.tile([1, QW], F32, tag="den",
                                       name=f"den_{qc}_{hp}_{s_}")
                        nc.vector.tensor_copy(den[0:1, :],
                                              outps[s_][HD:HD + 1, :])
                        rec = prc.tile([1, QW], F32R, tag="rec",
                                       name=f"rec_{qc}_{hp}_{s_}")
                        nc.vector._custom_dve(
                            RECIPROCAL_APPROX_FAST,
                            out=rec[0:1, :],
                            in0=den[0:1, :],
                            s0=rc["s0"], s1=rc["s1"], imm2=rc["imm2"],
                        )
                        bct = ps_wv.tile([HD, QW], F32, tag="wv",
                                         name=f"bct_{qc}_{hp}_{s_}")
                        nc.tensor.matmul(bct[:], cr[0:1, 0:HD], rec[0:1, :],
                                         start=True, stop=True)
                        bc = pbc.tile([HD, QW], F32R, tag="bc",
                                      name=f"bc_{qc}_{hp}_{s_}")
                        nc.vector.tensor_copy(bc[:], bct[:])
                        nc.vector.tensor_mul(
                            outtc[ho:ho + HD, hp, :], outps[s_][0:HD, :], bc[:],
                        )
                    pop_filler()

                # P4 of this chunk becomes filler for the next chunk
                for sbl in range(4):
                    p4q.append(
                        lambda outtc=outtc, qc=qc, sbl=sbl:
                            p4_unit(outtc, qc, sbl, nc.vector.tensor_copy))

            while p4q or bulk:
                pop_filler()

    nc.compile()
    return nc


_NC_CACHE = None


def _get_nc():
    global _NC_CACHE
    if _NC_CACHE is None:
        _NC_CACHE = build()
    return _NC_CACHE


def make_in_maps(x, Wq, Wk, Wv, Wr):
    import ml_dtypes
    bf16 = ml_dtypes.bfloat16

    x = np.ascontiguousarray(x, dtype=np.float32)
    Wq = np.asarray(Wq, dtype=np.float32)
    Wk = np.asarray(Wk, dtype=np.float32)
    Wv = np.asarray(Wv, dtype=np.float32)
    Wr = np.asarray(Wr, dtype=np.float32)

    cb = np.zeros((128, 640), dtype=np.float32)
    cb[:, 0:128] = np.triu(np.ones((128, 128), dtype=np.float32))
    cb[:, 128:256] = 1.0
    cb = cb.astype(bf16)
    cr = np.ones((1, 64), dtype=np.float32)

    def swz(w):  # [1024, 512] -> [p, hp, e, n]
        return np.ascontiguousarray(
            w.reshape(NEC, 128, 4, 128).transpose(1, 2, 0, 3).astype(bf16))

    in_maps = []
    for core in range(NCORES):
        b, g = divmod(core, 2)
        hs = slice(g * GW, (g + 1) * GW)
        in_maps.append({
            "xt": np.ascontiguousarray(x[b].T.astype(bf16)),
            "wq": swz(Wq[:, hs] * SCALE),
            "wk": swz(Wk[:, hs]),
            "wv": np.ascontiguousarray(
                Wv[:, hs].reshape(NEC, 128, GW).transpose(1, 0, 2).astype(bf16)),
            "wr": np.ascontiguousarray(
                Wr[hs, :].reshape(4, 128, EMB).transpose(1, 0, 2).astype(bf16)),
            "cb": cb,
            "cr": cr,
        })
    return in_maps


def kernel(x, Wq, Wk, Wv, Wr):
    in_maps = make_in_maps(x, Wq, Wk, Wv, Wr)
    nc = _get_nc()
    res = run_bass_kernel_spmd(nc, in_maps, core_ids=list(range(NCORES)))

    y = np.empty((B, S, EMB), dtype=np.float32)
    for b in range(B):
        y[b] = res.results[2 * b]["y"] + res.results[2 * b + 1]["y"]
    return y
